# revision 19
# baseline (speedup 1.0000x reference)
# Bass/Trainium2 kernel for nn_AA2_Module_75359496175785 (sparse_attention).
#
# Math (per batch item b; x: (C,N) with C=128, N=H*W=16384):
#   q  = Wq x + bq;  k_g = Wk_g pool(x) + bk_g   (pooling commutes with 1x1 conv)
#   e_g = q^T k_g;   a_g = softmax(alpha_g e_g, axis=keys)
#   out = gamma0 k_0 a_0^T + x + gamma1 k_1 a_1^T
#
# Keys-partition layout throughout; per 512-column group of n:
#   PE : energy = k_bf^T q_bf          -> psum (128 keys, 512)        [bf16]
#   ACT: exp(energy)                   -> sbuf bf16  (1024-wide ops)
#   PE : blockdiag-ones^T exp          -> psum s_bcast (sums replicated
#                                         across the 64 rows of each key group)
#   DVE: r = approx 1/s_bcast          -> sbuf f32
#   G/D: attn = exp * r (split cols)   -> sbuf bf16
#   PE : u = kT^T attn (+) I^T x_bf    -> psum (128 c, 512)
#   A/D: copy u -> sbuf f32 (split cols); DMA out.
#
# alpha and the 1/256 pooling mean fold into the Wk weights host-side;
# gamma/alpha folds into the per-key-row scale when building kT.
# Sharding: pure data parallel, batch b -> core b (B=8, 8 cores).

import numpy as np

B, C, H, W = 8, 128, 128, 128
N = H * W
PP = 8
NKEYS = 64
CHUNK = 2048      # phase-0 dma chunk = 16 rows of H
NCHUNK = N // CHUNK
GRP = 512         # phase-1 group (one psum bank of f32)
NGRP = N // GRP
TT_G = 768        # columns (of each 1024-pair) whose attn-multiply runs on gpsimd
FIN_A = 640       # columns (of each 1024-pair) whose final copy runs on ACT

_CACHE = {}


def _build_nc():
    import concourse.bass as bass  # noqa: F401
    from concourse import bacc, mybir
    import concourse.tile as tile

    f32 = mybir.dt.float32
    bf16 = mybir.dt.bfloat16
    AF = mybir.ActivationFunctionType

    nc = bacc.Bacc(None, target_bir_lowering=False)

    x_d = nc.dram_tensor("x", [C, N], f32, kind="ExternalInput")
    # packed weights: bf16 [wq | idb | ones], f32 [wkT | wk1T | bq bk bk1 gvec]
    wb_d = nc.dram_tensor("wb", [C, 3 * C], bf16, kind="ExternalInput")
    wf_d = nc.dram_tensor("wf", [C, 2 * C + 4], f32, kind="ExternalInput")
    out_d = nc.dram_tensor("out", [C, N], f32, kind="ExternalOutput")

    with tile.TileContext(nc) as tc:
        with (
            tc.tile_pool(name="const", bufs=1) as const,
            tc.tile_pool(name="big", bufs=1) as big,
            tc.tile_pool(name="pool_r1", bufs=2) as pool_r1,
            tc.tile_pool(name="expp", bufs=4) as expp,
            tc.tile_pool(name="rp", bufs=4) as rp,
            tc.tile_pool(name="attnp", bufs=4) as attnp,
            tc.tile_pool(name="outp", bufs=6) as outp,
        ):
            wb = const.tile([C, 3 * C], bf16)
            wf = const.tile([C, 2 * C + 4], f32)
            nc.sync.dma_start(wb[:], wb_d[:])
            nc.sync.dma_start(wf[:], wf_d[:])
            wq_bf = wb[:, 0:C]  # Wq natural (c-in rows)
            idb = wb[:, C:2 * C]
            ones_bd = wb[:, 2 * C:3 * C]
            wkT = wf[:, 0:C]
            wk1T = wf[:, C:2 * C]
            bq = wf[:, 2 * C:2 * C + 1]
            bk = wf[:, 2 * C + 1:2 * C + 2]
            bk1 = wf[:, 2 * C + 2:2 * C + 3]
            gvec = wf[:, 2 * C + 3:2 * C + 4]

            x_sb = big.tile([C, N], f32)
            x_bf = big.tile([C, N], bf16)
            xp = big.tile([C, NKEYS], f32)
            k_bf = big.tile([C, 2 * NKEYS], bf16)
            kT = big.tile([C, C], bf16)
            m_bf = big.tile([C, C], bf16)
            ebias = big.tile([C, 1], f32)

            # ---- phase 0: stream x, cast to bf16, q-conv, pooling ----
            ph0 = tc.tile_pool(name="ps0", bufs=2, space="PSUM")
            ps0 = ph0.__enter__()
            for c in range(NCHUNK):
                csl = bass.ts(c, CHUNK)
                nc.sync.dma_start(x_sb[:, csl], x_d[:, csl])
                nc.vector.tensor_copy(
                    x_bf[:, bass.ds(c * CHUNK, 2 * GRP)],
                    x_sb[:, bass.ds(c * CHUNK, 2 * GRP)],
                )
                nc.scalar.copy(
                    x_bf[:, bass.ds(c * CHUNK + 2 * GRP, GRP)],
                    x_sb[:, bass.ds(c * CHUNK + 2 * GRP, GRP)],
                )
                nc.gpsimd.tensor_copy(
                    x_bf[:, bass.ds(c * CHUNK + 3 * GRP, GRP)],
                    x_sb[:, bass.ds(c * CHUNK + 3 * GRP, GRP)],
                )
                # pooling (sums; /256 folded into wk weights host-side)
                if c % 4 == 1:
                    scratch = pool_r1.tile([C, 16, 16], f32, tag="scr")
                    for pj in range(PP):
                        blk = x_sb[:, csl].rearrange(
                            "p (h pj w) -> p pj h w", h=16, pj=PP, w=16
                        )[:, pj, :, :]
                        nc.scalar.activation(
                            scratch[:], blk, AF.Copy,
                            accum_out=xp[:, c * PP + pj:c * PP + pj + 1],
                        )
                else:
                    xc = x_sb[:, csl].rearrange(
                        "p (h pj w) -> p pj h w", h=16, pj=PP, w=16
                    )
                    nc.vector.tensor_reduce(
                        xp[:, c * PP:(c + 1) * PP], xc,
                        axis=mybir.AxisListType.XY, op=mybir.AluOpType.add,
                    )

            # ---- mid: keys ----
            k_ps = ps0.tile([C, 2 * NKEYS], f32, tag="qps")
            nc.tensor.matmul(k_ps[:, 0:NKEYS], wkT, xp[:], start=True, stop=True)
            nc.tensor.matmul(k_ps[:, NKEYS:], wk1T, xp[:], start=True, stop=True)
            nc.scalar.activation(
                k_bf[:, 0:NKEYS], k_ps[:, 0:NKEYS], AF.Identity, bias=bk, scale=1.0,
            )
            nc.scalar.activation(
                k_bf[:, NKEYS:], k_ps[:, NKEYS:], AF.Identity, bias=bk1, scale=1.0,
            )
            kT_ps = ps0.tile([C, C], bf16, tag="qps")
            nc.tensor.transpose(kT_ps[:], k_bf[:], idb)
            nc.scalar.activation(kT[:], kT_ps[:], AF.Copy, scale=gvec)
            # m = Wq^T k_cat (energy folds the q-conv); e0 = k_cat^T bq (per-key
            # energy bias from bq, applied inside exp)
            m_ps = ps0.tile([C, C], f32, tag="qps")
            nc.tensor.matmul(m_ps[:], wq_bf, k_bf[:], start=True, stop=True)
            nc.scalar.activation(m_bf[:], m_ps[:], AF.Copy)
            bq_bf = rp.tile([C, 1], bf16, tag="bqbf")
            nc.vector.tensor_copy(bq_bf[:], bq)
            e_ps0 = ps0.tile([C, 1], f32, tag="ebps")
            nc.tensor.matmul(e_ps0[:], k_bf[:], bq_bf[:], start=True, stop=True)
            nc.vector.tensor_copy(ebias[:], e_ps0[:])
            ph0.__exit__(None, None, None)
            ph1 = tc.tile_pool(name="ps1", bufs=2, space="PSUM")
            ps1 = ph1.__enter__()
            ph1b = tc.tile_pool(name="ps1b", bufs=1, space="PSUM")
            ps1b = ph1b.__enter__()

            # ---- phase 1: two 512-groups ("pair") per iteration ----
            for gp in range(NGRP // 2):
                psl = bass.ds(gp * 2 * GRP, 2 * GRP)
                e_ps = ps1.tile([C, 2 * GRP], f32, tag="eps")
                for h in range(2):
                    nc.tensor.matmul(
                        e_ps[:, h * GRP:(h + 1) * GRP],
                        m_bf[:],
                        x_bf[:, bass.ds((gp * 2 + h) * GRP, GRP)],
                        start=True, stop=True,
                    )
                exp_sb = expp.tile([C, 2 * GRP], bf16)
                nc.scalar.activation(exp_sb[:], e_ps[:], AF.Exp, bias=ebias[:, 0:1], scale=1.0)
                r_sb = rp.tile([C, 2 * GRP], f32)
                for h in range(2):
                    s_ps = ps1.tile([C, GRP], f32, tag="sps")
                    nc.tensor.matmul(
                        s_ps[:], ones_bd,
                        exp_sb[:, h * GRP:(h + 1) * GRP],
                        start=True, stop=True,
                    )
                    nc.vector.reciprocal_approx_fast(
                        out=r_sb[:, h * GRP:(h + 1) * GRP], in_=s_ps[:]
                    )
                attn = attnp.tile([C, 2 * GRP], bf16)
                nc.gpsimd.tensor_mul(
                    attn[:, 0:TT_G], exp_sb[:, 0:TT_G], r_sb[:, 0:TT_G]
                )
                nc.vector.tensor_mul(
                    attn[:, TT_G:], exp_sb[:, TT_G:], r_sb[:, TT_G:]
                )
                u_ps = ps1b.tile([C, 2 * GRP], f32, tag="ups")
                nc.tensor.matmul(
                    u_ps[:, 0:GRP], kT[:], attn[:, 0:GRP],
                    start=True, stop=False,
                )
                nc.tensor.matmul(
                    u_ps[:, 0:GRP], idb,
                    x_bf[:, bass.ds(gp * 2 * GRP, GRP)],
                    start=False, stop=True,
                )
                nc.tensor.matmul(
                    u_ps[:, GRP:], kT[:], attn[:, GRP:],
                    start=True, stop=True,
                )
                o_sb = outp.tile([C, 2 * GRP], f32)
                nc.scalar.activation(o_sb[:, 0:GRP], u_ps[:, 0:GRP], AF.Copy)
                nc.vector.scalar_tensor_tensor(
                    out=o_sb[:, GRP:],
                    in0=u_ps[:, GRP:],
                    scalar=1.0,
                    in1=x_sb[:, bass.ds((gp * 2 + 1) * GRP, GRP)],
                    op0=mybir.AluOpType.mult,
                    op1=mybir.AluOpType.add,
                )
                nc.sync.dma_start(out_d[:, psl], o_sb[:])
            ph1b.__exit__(None, None, None)
            ph1.__exit__(None, None, None)

    nc.compile()
    return nc


def _get_nc():
    if "nc" not in _CACHE:
        _CACHE["nc"] = _build_nc()
    return _CACHE["nc"]


def _make_in_maps(x, Wq, bq, Wk, bk, Wk1, bk1, gamma, gamma1, aphal, aphal1):
    a0 = float(np.asarray(aphal).reshape(-1)[0])
    a1 = float(np.asarray(aphal1).reshape(-1)[0])
    g0 = float(np.asarray(gamma).reshape(-1)[0])
    g1 = float(np.asarray(gamma1).reshape(-1)[0])

    f = np.float32
    eye = np.eye(C, dtype=f)
    ones_bd = np.kron(np.eye(2, dtype=f), np.ones((NKEYS, NKEYS), f))
    wb = np.concatenate([Wq.astype(f), eye, ones_bd], axis=1).astype("bfloat16")
    gvec = np.concatenate(
        [np.full((NKEYS, 1), g0 / a0, f), np.full((NKEYS, 1), g1 / a1, f)]
    )
    wf = np.concatenate(
        [
            (Wk.T * (a0 / 256.0)).astype(f),
            (Wk1.T * (a1 / 256.0)).astype(f),
            bq.reshape(C, 1).astype(f),
            (bk.reshape(C, 1) * a0).astype(f),
            (bk1.reshape(C, 1) * a1).astype(f),
            gvec,
        ],
        axis=1,
    )
    wb = np.ascontiguousarray(wb)
    wf = np.ascontiguousarray(wf)
    in_maps = []
    for b in range(B):
        in_maps.append({
            "x": np.ascontiguousarray(x[b].reshape(C, N), dtype=f),
            "wb": wb,
            "wf": wf,
        })
    return in_maps


def kernel(x, Wq, bq, Wk, bk, Wk1, bk1, gamma, gamma1, aphal, aphal1, **_):
    import ml_dtypes  # noqa: F401
    from concourse.bass_utils import run_bass_kernel_spmd

    nc = _get_nc()
    in_maps = _make_in_maps(
        np.asarray(x), np.asarray(Wq), np.asarray(bq), np.asarray(Wk),
        np.asarray(bk), np.asarray(Wk1), np.asarray(bk1), np.asarray(gamma),
        np.asarray(gamma1), np.asarray(aphal), np.asarray(aphal1),
    )
    res = None
    last_exc = None
    for _attempt in range(3):
        try:
            res = run_bass_kernel_spmd(nc, in_maps, core_ids=list(range(B)))
            break
        except Exception as e:  # transient NRT_EXEC_UNIT_UNRECOVERABLE faults
            last_exc = e
            import time as _time
            _time.sleep(2.0)
    if res is None:
        raise last_exc
    out = np.stack([res.results[b]["out"].reshape(C, H, W) for b in range(B)])
    return out.astype(np.float32)


# revision 20
# speedup vs baseline: 1.0990x; 1.0990x over previous
# Bass/Trainium2 kernel for nn_AA2_Module_75359496175785 (sparse_attention).
#
# Math (per batch item b; x: (C,N) with C=128, N=H*W=16384):
#   q  = Wq x + bq;  k_g = Wk_g pool(x) + bk_g   (pooling commutes with 1x1 conv)
#   e_g = q^T k_g;   a_g = softmax(alpha_g e_g, axis=keys)
#   out = gamma0 k_0 a_0^T + x + gamma1 k_1 a_1^T
#
# Keys-partition layout throughout; per 512-column group of n:
#   PE : energy = k_bf^T q_bf          -> psum (128 keys, 512)        [bf16]
#   ACT: exp(energy)                   -> sbuf bf16  (1024-wide ops)
#   PE : blockdiag-ones^T exp          -> psum s_bcast (sums replicated
#                                         across the 64 rows of each key group)
#   DVE: r = approx 1/s_bcast          -> sbuf f32
#   G/D: attn = exp * r (split cols)   -> sbuf bf16
#   PE : u = kT^T attn (+) I^T x_bf    -> psum (128 c, 512)
#   A/D: copy u -> sbuf f32 (split cols); DMA out.
#
# alpha and the 1/256 pooling mean fold into the Wk weights host-side;
# gamma/alpha folds into the per-key-row scale when building kT.
# Sharding: pure data parallel, batch b -> core b (B=8, 8 cores).

import numpy as np

B, C, H, W = 8, 128, 128, 128
N = H * W
PP = 8
NKEYS = 64
CHUNK = 2048      # phase-0 dma chunk = 16 rows of H
NCHUNK = N // CHUNK
GRP = 512         # phase-1 group (one psum bank of f32)
NGRP = N // GRP
TT_G = 768        # columns (of each 1024-pair) whose attn-multiply runs on gpsimd
FIN_A = 640       # columns (of each 1024-pair) whose final copy runs on ACT

_CACHE = {}


def _build_nc():
    import concourse.bass as bass  # noqa: F401
    from concourse import bacc, mybir
    import concourse.tile as tile

    f32 = mybir.dt.float32
    bf16 = mybir.dt.bfloat16
    AF = mybir.ActivationFunctionType

    nc = bacc.Bacc(None, target_bir_lowering=False)

    x_d = nc.dram_tensor("x", [C, N], f32, kind="ExternalInput")
    # packed weights: bf16 [wq | idb | ones], f32 [wkT | wk1T | bq bk bk1 gvec]
    wb_d = nc.dram_tensor("wb", [C, 3 * C], bf16, kind="ExternalInput")
    wf_d = nc.dram_tensor("wf", [C, 2 * C + 4], f32, kind="ExternalInput")
    out_d = nc.dram_tensor("out", [C, N], f32, kind="ExternalOutput")

    with tile.TileContext(nc) as tc:
        with (
            tc.tile_pool(name="const", bufs=1) as const,
            tc.tile_pool(name="big", bufs=1) as big,
            tc.tile_pool(name="pool_r1", bufs=2) as pool_r1,
            tc.tile_pool(name="expp", bufs=4) as expp,
            tc.tile_pool(name="rp", bufs=4) as rp,
            tc.tile_pool(name="attnp", bufs=4) as attnp,
            tc.tile_pool(name="outp", bufs=6) as outp,
        ):
            wb = const.tile([C, 3 * C], bf16)
            wf = const.tile([C, 2 * C + 4], f32)
            nc.sync.dma_start(wb[:], wb_d[:])
            nc.sync.dma_start(wf[:], wf_d[:])
            wq_bf = wb[:, 0:C]  # Wq natural (c-in rows)
            idb = wb[:, C:2 * C]
            ones_bd = wb[:, 2 * C:3 * C]
            wkT = wf[:, 0:C]
            wk1T = wf[:, C:2 * C]
            bq = wf[:, 2 * C:2 * C + 1]
            bk = wf[:, 2 * C + 1:2 * C + 2]
            bk1 = wf[:, 2 * C + 2:2 * C + 3]
            gvec = wf[:, 2 * C + 3:2 * C + 4]

            x_sb = big.tile([C, N], f32)
            x_bf = big.tile([C, N], bf16)
            xp = big.tile([C, NKEYS], f32)
            k_bf = big.tile([C, 2 * NKEYS], bf16)
            kT = big.tile([C, C], bf16)
            m_bf = big.tile([C, C], bf16)
            ebias = big.tile([C, 1], f32)

            # ---- phase 0: stream x, cast to bf16, q-conv, pooling ----
            ph0 = tc.tile_pool(name="ps0", bufs=2, space="PSUM")
            ps0 = ph0.__enter__()
            for c in range(NCHUNK):
                csl = bass.ts(c, CHUNK)
                nc.sync.dma_start(x_sb[:, csl], x_d[:, csl])
                nc.vector.tensor_copy(
                    x_bf[:, bass.ds(c * CHUNK, 2 * GRP)],
                    x_sb[:, bass.ds(c * CHUNK, 2 * GRP)],
                )
                nc.scalar.copy(
                    x_bf[:, bass.ds(c * CHUNK + 2 * GRP, GRP)],
                    x_sb[:, bass.ds(c * CHUNK + 2 * GRP, GRP)],
                )
                nc.gpsimd.tensor_copy(
                    x_bf[:, bass.ds(c * CHUNK + 3 * GRP, GRP)],
                    x_sb[:, bass.ds(c * CHUNK + 3 * GRP, GRP)],
                )
                # pooling (sums; /256 folded into wk weights host-side)
                if c % 4 == 1:
                    scratch = pool_r1.tile([C, 16, 16], f32, tag="scr")
                    for pj in range(PP):
                        blk = x_sb[:, csl].rearrange(
                            "p (h pj w) -> p pj h w", h=16, pj=PP, w=16
                        )[:, pj, :, :]
                        nc.scalar.activation(
                            scratch[:], blk, AF.Copy,
                            accum_out=xp[:, c * PP + pj:c * PP + pj + 1],
                        )
                else:
                    xc = x_sb[:, csl].rearrange(
                        "p (h pj w) -> p pj h w", h=16, pj=PP, w=16
                    )
                    nc.vector.tensor_reduce(
                        xp[:, c * PP:(c + 1) * PP], xc,
                        axis=mybir.AxisListType.XY, op=mybir.AluOpType.add,
                    )

            # ---- mid: keys ----
            k_ps = ps0.tile([C, 2 * NKEYS], f32, tag="qps")
            nc.tensor.matmul(k_ps[:, 0:NKEYS], wkT, xp[:], start=True, stop=True)
            nc.tensor.matmul(k_ps[:, NKEYS:], wk1T, xp[:], start=True, stop=True)
            nc.scalar.activation(
                k_bf[:, 0:NKEYS], k_ps[:, 0:NKEYS], AF.Identity, bias=bk, scale=1.0,
            )
            nc.scalar.activation(
                k_bf[:, NKEYS:], k_ps[:, NKEYS:], AF.Identity, bias=bk1, scale=1.0,
            )
            kT_ps = ps0.tile([C, C], bf16, tag="qps")
            nc.tensor.transpose(kT_ps[:], k_bf[:], idb)
            nc.scalar.activation(kT[:], kT_ps[:], AF.Copy, scale=gvec)
            # m = Wq^T k_cat (energy folds the q-conv); e0 = k_cat^T bq (per-key
            # energy bias from bq, applied inside exp)
            m_ps = ps0.tile([C, C], f32, tag="qps")
            nc.tensor.matmul(m_ps[:], wq_bf, k_bf[:], start=True, stop=True)
            nc.scalar.activation(m_bf[:], m_ps[:], AF.Copy)
            bq_bf = rp.tile([C, 1], bf16, tag="bqbf")
            nc.vector.tensor_copy(bq_bf[:], bq)
            e_ps0 = ps0.tile([C, 1], f32, tag="ebps")
            nc.tensor.matmul(e_ps0[:], k_bf[:], bq_bf[:], start=True, stop=True)
            nc.vector.tensor_copy(ebias[:], e_ps0[:])
            ph0.__exit__(None, None, None)
            ph1 = tc.tile_pool(name="ps1", bufs=2, space="PSUM")
            ps1 = ph1.__enter__()
            ph1b = tc.tile_pool(name="ps1b", bufs=1, space="PSUM")
            ps1b = ph1b.__enter__()

            # ---- phase 1: two 512-groups ("pair") per iteration ----
            for gp in range(NGRP // 2):
                psl = bass.ds(gp * 2 * GRP, 2 * GRP)
                e_ps = ps1b.tile([C, 2 * GRP], f32, tag="eps")
                for h in range(2):
                    nc.tensor.matmul(
                        e_ps[:, h * GRP:(h + 1) * GRP],
                        m_bf[:],
                        x_bf[:, bass.ds((gp * 2 + h) * GRP, GRP)],
                        start=True, stop=True,
                    )
                exp_sb = expp.tile([C, 2 * GRP], bf16)
                nc.scalar.activation(exp_sb[:], e_ps[:], AF.Exp, bias=ebias[:, 0:1], scale=1.0)
                r_sb = rp.tile([C, 2 * GRP], f32)
                for h in range(2):
                    s_ps = ps1.tile([C, GRP], f32, tag="sps")
                    nc.tensor.matmul(
                        s_ps[:], ones_bd,
                        exp_sb[:, h * GRP:(h + 1) * GRP],
                        start=True, stop=True,
                    )
                    nc.vector.reciprocal_approx_fast(
                        out=r_sb[:, h * GRP:(h + 1) * GRP], in_=s_ps[:]
                    )
                attn = attnp.tile([C, 2 * GRP], bf16)
                nc.gpsimd.tensor_mul(
                    attn[:, 0:TT_G], exp_sb[:, 0:TT_G], r_sb[:, 0:TT_G]
                )
                nc.vector.tensor_mul(
                    attn[:, TT_G:], exp_sb[:, TT_G:], r_sb[:, TT_G:]
                )
                u_ps = ps1.tile([C, 2 * GRP], f32, tag="ups")
                nc.tensor.matmul(
                    u_ps[:, 0:GRP], kT[:], attn[:, 0:GRP],
                    start=True, stop=False,
                )
                nc.tensor.matmul(
                    u_ps[:, 0:GRP], idb,
                    x_bf[:, bass.ds(gp * 2 * GRP, GRP)],
                    start=False, stop=True,
                )
                nc.tensor.matmul(
                    u_ps[:, GRP:], kT[:], attn[:, GRP:],
                    start=True, stop=True,
                )
                o_sb = outp.tile([C, 2 * GRP], f32)
                nc.scalar.activation(o_sb[:, 0:GRP], u_ps[:, 0:GRP], AF.Copy)
                nc.vector.scalar_tensor_tensor(
                    out=o_sb[:, GRP:],
                    in0=u_ps[:, GRP:],
                    scalar=1.0,
                    in1=x_sb[:, bass.ds((gp * 2 + 1) * GRP, GRP)],
                    op0=mybir.AluOpType.mult,
                    op1=mybir.AluOpType.add,
                )
                nc.sync.dma_start(out_d[:, psl], o_sb[:])
            ph1b.__exit__(None, None, None)
            ph1.__exit__(None, None, None)

    nc.compile()
    return nc


def _get_nc():
    if "nc" not in _CACHE:
        _CACHE["nc"] = _build_nc()
    return _CACHE["nc"]


def _make_in_maps(x, Wq, bq, Wk, bk, Wk1, bk1, gamma, gamma1, aphal, aphal1):
    a0 = float(np.asarray(aphal).reshape(-1)[0])
    a1 = float(np.asarray(aphal1).reshape(-1)[0])
    g0 = float(np.asarray(gamma).reshape(-1)[0])
    g1 = float(np.asarray(gamma1).reshape(-1)[0])

    f = np.float32
    eye = np.eye(C, dtype=f)
    ones_bd = np.kron(np.eye(2, dtype=f), np.ones((NKEYS, NKEYS), f))
    wb = np.concatenate([Wq.astype(f), eye, ones_bd], axis=1).astype("bfloat16")
    gvec = np.concatenate(
        [np.full((NKEYS, 1), g0 / a0, f), np.full((NKEYS, 1), g1 / a1, f)]
    )
    wf = np.concatenate(
        [
            (Wk.T * (a0 / 256.0)).astype(f),
            (Wk1.T * (a1 / 256.0)).astype(f),
            bq.reshape(C, 1).astype(f),
            (bk.reshape(C, 1) * a0).astype(f),
            (bk1.reshape(C, 1) * a1).astype(f),
            gvec,
        ],
        axis=1,
    )
    wb = np.ascontiguousarray(wb)
    wf = np.ascontiguousarray(wf)
    in_maps = []
    for b in range(B):
        in_maps.append({
            "x": np.ascontiguousarray(x[b].reshape(C, N), dtype=f),
            "wb": wb,
            "wf": wf,
        })
    return in_maps


def kernel(x, Wq, bq, Wk, bk, Wk1, bk1, gamma, gamma1, aphal, aphal1, **_):
    import ml_dtypes  # noqa: F401
    from concourse.bass_utils import run_bass_kernel_spmd

    nc = _get_nc()
    in_maps = _make_in_maps(
        np.asarray(x), np.asarray(Wq), np.asarray(bq), np.asarray(Wk),
        np.asarray(bk), np.asarray(Wk1), np.asarray(bk1), np.asarray(gamma),
        np.asarray(gamma1), np.asarray(aphal), np.asarray(aphal1),
    )
    res = None
    last_exc = None
    for _attempt in range(3):
        try:
            res = run_bass_kernel_spmd(nc, in_maps, core_ids=list(range(B)))
            break
        except Exception as e:  # transient NRT_EXEC_UNIT_UNRECOVERABLE faults
            last_exc = e
            import time as _time
            _time.sleep(2.0)
    if res is None:
        raise last_exc
    out = np.stack([res.results[b]["out"].reshape(C, H, W) for b in range(B)])
    return out.astype(np.float32)
